# revision 39
# baseline (speedup 1.0000x reference)
"""Multi-head masked attention on 8 Trainium2 NeuronCores.

Sharding: data-parallel over batch (B=2 -> 2 groups of 4 cores),
tensor-parallel over heads within a group (16 heads -> 4 heads/core).
Each core computes q/k/v projections for its 4 heads (column-sharded),
causal attention in the transposed (S^T) domain, and a row-sharded
partial o-projection. The host sums the 4 partials per batch element
and adds the output bias.

Pipeline structure: projection / o-projection matmuls are emitted as
fill units interleaved into the attention QK->exp->PV stream so the PE
never idles long enough for the HAM clock gate to re-throttle. PV uses
a merged [V | ones] stationary per head: one N-wide stream yields both
y^T (rows 0-63) and the softmax denominator replicated across rows
64-127, so the denominator costs no extra PE stream and normalization
is a partition-offset DVE multiply. The prologue streams x^T per
c-chunk so the first projection starts as soon as its slice lands.

Self-contained: hardcodes shapes B=2, T=2048, C=1024, H=16, Dh=64.
"""

import sys

sys.path.insert(0, "/opt/trn_rl_repo")

import numpy as np

import concourse.bass as bass
import concourse.tile as tile
import concourse.mybir as mybir
from concourse import bacc
from concourse.bass import ts, ds
from concourse.masks import make_identity, make_lower_triangular

F32 = mybir.dt.float32
BF16 = mybir.dt.bfloat16
AF = mybir.ActivationFunctionType
ALU = mybir.AluOpType

B, T, C = 2, 2048, 1024
H, DH = 16, 64
HPC = 4            # heads per core
DQC = HPC * DH     # 256 projected dims per core
N_CORES = 8
NEG = -1.0e30

TC = T // 128      # 16 s-chunks of 128
CC = C // 128      # 8 c-chunks
TJ = T // 512      # 4 t-chunks of 512
SCALE = 1.0 / np.sqrt(DH)


def build_program():
    nc = bacc.Bacc("TRN2", target_bir_lowering=False, debug=False)

    xbT = nc.dram_tensor("xbT", [C, T], BF16, kind="ExternalInput")
    wq = nc.dram_tensor("wq", [2, C, 128], BF16, kind="ExternalInput")
    wk = nc.dram_tensor("wk", [2, C, 128], BF16, kind="ExternalInput")
    wv = nc.dram_tensor("wv", [C, DQC], BF16, kind="ExternalInput")
    wo = nc.dram_tensor("wo", [DQC, C], BF16, kind="ExternalInput")
    bq = nc.dram_tensor("bq", [DQC], F32, kind="ExternalInput")
    bk = nc.dram_tensor("bk", [DQC], F32, kind="ExternalInput")
    bv = nc.dram_tensor("bv", [DQC], F32, kind="ExternalInput")
    out = nc.dram_tensor("out", [T, C], BF16, kind="ExternalOutput")

    with tile.TileContext(nc) as tc:
        with (
            tc.tile_pool(name="persist", bufs=1) as pp,
            tc.tile_pool(name="psb", bufs=5) as pexp,
            tc.tile_pool(name="otp", bufs=3) as pot,
            tc.tile_pool(name="rcp", bufs=2) as prc,
            tc.tile_pool(name="ps_s", bufs=2, space="PSUM") as ps_s,
            tc.tile_pool(name="ps_y", bufs=1, space="PSUM") as ps_y,
            tc.tile_pool(name="ps_d", bufs=1, space="PSUM") as ps_d,
            tc.tile_pool(name="ps_po", bufs=2, space="PSUM") as ps_po,
        ):
            # ---- persistent sbuf tensors -------------------------------
            xT = pp.tile([128, CC, T], BF16, tag="xT")
            qT = pp.tile([128, 2, T], BF16, tag="qT")   # [d-pair, hp, t]
            kT = pp.tile([128, 2, T], BF16, tag="kT")
            # [s, sc, h, d]; slot HPC holds all-ones denominator columns
            vA = pp.tile([128, TC, HPC + 1, DH], BF16, tag="vA")
            yT = pp.tile([128, 2, T], BF16, tag="yT")
            wq_sb = pp.tile([128, CC, DQC], BF16, tag="wq")
            wk_sb = pp.tile([128, CC, DQC], BF16, tag="wk")
            wv_sb = pp.tile([128, CC, DQC], BF16, tag="wv")
            wo_sb = pp.tile([128, 2, C], BF16, tag="wo")
            identb = pp.tile([128, 128], BF16, tag="identb")
            atrif = pp.tile([128, 128], BF16, tag="atrif")
            atrif2 = pp.tile([128, 2, 128], F32, tag="atrif2")
            warm = pp.tile([128, 128], BF16, tag="warm")
            bqs = pp.tile([128, 2], F32, tag="bqs")
            bks = pp.tile([128, 2], F32, tag="bks")
            bvs = pp.tile([128, DQC], F32, tag="bvs")

            # ---- PE warmup: keep the array busy while DMA lands -------
            # (memset on DVE: the Pool engine's first instruction pays a
            # multi-us ucode bring-up that would delay the warm matmuls)
            nc.vector.memset(warm[:], 0.25)
            pw0 = ps_po.tile([128, 512], F32, tag="po")
            for i in range(36):
                nc.tensor.matmul(
                    pw0[:, :128], warm[:], warm[:],
                    start=True, stop=True, skip_group_check=True,
                )

            # ---- DMA: first-needed pieces first -----------------------
            xTr = xbT.ap().rearrange("(c p) t -> p c t", p=128)
            nc.sync.dma_start(
                wq_sb[:, :, 0:128],
                wq.ap()[0].rearrange("(c p) d -> p c d", p=128),
            )
            nc.sync.dma_start(
                wk_sb[:, :, 0:128],
                wk.ap()[0].rearrange("(c p) d -> p c d", p=128),
            )
            # biases early: the first projection drains need them
            nc.sync.dma_start(bqs[:], bq.ap().rearrange("(k p) -> p k", p=128))
            nc.vector.tensor_scalar_mul(bqs[:], bqs[:], SCALE)
            nc.sync.dma_start(bks[:], bk.ap().rearrange("(k p) -> p k", p=128))
            nc.sync.dma_start(
                bvs[0:1, :], bv.ap().rearrange("(o n) -> o n", o=1)
            )
            nc.gpsimd.partition_broadcast(bvs[:], bvs[0:1, :])
            # Input DMAs are spread across three rings (sync / gpsimd /
            # scalar HWDGE) — a single ring sustains only ~115 GB/s, so
            # serializing the ~6 MB of inputs on one ring starves the
            # prologue. Each ring gets its pieces in first-needed order.
            # sync ring: x^T chunk 0 low c-slices (Q(0,h0) starts at
            # slice-0 landing)
            for cc in range(CC // 2):
                nc.sync.dma_start(
                    xT[:, cc, ts(0, 512)], xTr[:, cc, ts(0, 512)]
                )
            # gpsimd ring: second-half weights, x0 high c-slices, wv, x3
            nc.gpsimd.dma_start(
                wq_sb[:, :, 128:256],
                wq.ap()[1].rearrange("(c p) d -> p c d", p=128),
            )
            nc.gpsimd.dma_start(
                wk_sb[:, :, 128:256],
                wk.ap()[1].rearrange("(c p) d -> p c d", p=128),
            )
            for cc in range(CC // 2, CC):
                nc.gpsimd.dma_start(
                    xT[:, cc, ts(0, 512)], xTr[:, cc, ts(0, 512)]
                )
            nc.gpsimd.dma_start(
                wv_sb[:], wv.ap().rearrange("(c p) d -> p c d", p=128)
            )
            nc.gpsimd.dma_start(
                xT[:, :, ts(3, 512)], xTr[:, :, ts(3, 512)]
            )
            # scalar ring: x1, x2, wo
            for tj in range(1, 3):
                nc.scalar.dma_start(
                    xT[:, :, ts(tj, 512)], xTr[:, :, ts(tj, 512)]
                )
            nc.scalar.dma_start(
                wo_sb[:], wo.ap().rearrange("(k p) n -> p k n", p=128)
            )

            # constants
            make_identity(nc, identb[:])
            # atrif[p, f] = NEG where f < p (kills s>t in a diagonal block)
            make_lower_triangular(nc, atrif[:], val=NEG, diag=False)
            # f32 copy replicated for both head-halves (DVE mask add)
            nc.vector.tensor_copy(atrif2[:, 0, :], atrif[:])
            nc.vector.tensor_copy(atrif2[:, 1, :], atrif[:])
            # denominator ones-columns (slot HPC of vA)
            nc.vector.memset(vA[:, :, HPC, :], 1.0)

            # ---- work units (each emits a short burst of PE work) -----
            def qk_proj_unit(w_sb, b_sb, dstT, tj, hp, pre_scale):
                def emit():
                    pq = ps_po.tile([128, 512], F32, tag="po")
                    for cc in range(CC):
                        nc.tensor.matmul(
                            pq[:],
                            w_sb[:, cc, ts(hp, 128)],
                            xT[:, cc, ts(tj, 512)],
                            start=(cc == 0),
                            stop=(cc == CC - 1),
                        )
                    if pre_scale is not None:
                        nc.vector.tensor_scalar(
                            dstT[:, hp, ts(tj, 512)],
                            pq[:], pre_scale, b_sb[:, hp : hp + 1],
                            ALU.mult, ALU.add,
                        )
                    else:
                        nc.vector.tensor_scalar(
                            dstT[:, hp, ts(tj, 512)],
                            pq[:], b_sb[:, hp : hp + 1], None, ALU.add,
                        )
                return emit

            def v_proj_unit(sc):
                def emit():
                    pv = ps_po.tile([128, 512], F32, tag="po")
                    for cc in range(CC):
                        nc.tensor.matmul(
                            pv[:, :DQC],
                            xT[:, cc, ts(sc, 128)],
                            wv_sb[:, cc, :],
                            start=(cc == 0),
                            stop=(cc == CC - 1),
                        )
                    nc.vector.tensor_tensor(
                        vA[:, sc, 0:HPC, :],
                        pv[:, :DQC].rearrange("p (h d) -> p h d", d=DH),
                        bvs[:].rearrange("p (h d) -> p h d", d=DH),
                        ALU.add,
                    )
                return emit

            def o_proj_unit(tj, tt):
                # one t-block of the o-projection: out[t0:t0+128, :];
                # output DMAs alternate rings so the final stores overlap
                def emit():
                    t0 = 512 * tj + 128 * tt
                    ot = pot.tile([128, C], BF16, tag="ot")
                    for nb in range(2):
                        po = ps_po.tile([128, 512], F32, tag="po")
                        for kk in range(2):
                            nc.tensor.matmul(
                                po[:],
                                yT[:, kk, ds(t0, 128)],
                                wo_sb[:, kk, ts(nb, 512)],
                                start=(kk == 0),
                                stop=(kk == 1),
                            )
                        nc.vector.tensor_copy(ot[:, ts(nb, 512)], po[:])
                    eng = nc.sync if tt % 2 == 0 else nc.gpsimd
                    eng.dma_start(out.ap()[ds(t0, 128), :], ot[:])
                return emit

            fill = []

            def pop_fill(n=1):
                for _ in range(n):
                    if fill:
                        fill.pop(0)()

            # ---- attention sweep for one (tj, hp) head pair ------------
            def attn_sweep(tj, hp):
                n_sc = 4 * (tj + 1)
                psbs = {}

                def emit_qk(sc):
                    k = sc - 4 * tj
                    off = 128 * k if k > 0 else 0
                    pss = ps_s.tile([128, 2, 512], F32, tag="s")
                    for hi in range(2):
                        prow = slice(64 * hi, 64 * hi + 64)
                        nc.tensor.matmul(
                            pss[:, hi, off:512],
                            kT[prow, hp, ts(sc, 128)],
                            qT[prow, hp, ds(512 * tj + off, 512 - off)],
                            start=True,
                            stop=True,
                            tile_position=(64 * hi, 0),
                            skip_group_check=(k >= 0),
                        )
                    if k >= 0:
                        # causal mask on the diagonal block: DVE add keeps
                        # the PE free of identb weight-switch flushes
                        nc.vector.tensor_tensor(
                            pss[:, :, ds(off, 128)],
                            pss[:, :, ds(off, 128)],
                            atrif2[:],
                            ALU.add,
                        )
                    psb = pexp.tile([128, 2, 512], BF16, tag="p")
                    if off:
                        nc.scalar.activation(
                            psb[:, :, off:], pss[:, :, off:], AF.Exp
                        )
                    else:
                        nc.scalar.activation(psb[:], pss[:], AF.Exp)
                    psbs[sc] = psb

                def emit_pv(sc, ppy, ppd):
                    k = sc - 4 * tj
                    off = 128 * k if k > 0 else 0
                    psb = psbs.pop(sc)
                    first = sc == 0
                    last = sc == n_sc - 1
                    # y pair first (col groups 0-1 / 2-3 run concurrent),
                    # then the denominator pair
                    for hi in range(2):
                        nc.tensor.matmul(
                            ppy[ds(64 * hi, 64), off:512],
                            vA[:, sc, 2 * hp + hi, :],
                            psb[:, hi, off:512],
                            start=first,
                            stop=last,
                            tile_position=(0, 64 * hi),
                            skip_group_check=True,
                        )
                    for hi in range(2):
                        nc.tensor.matmul(
                            ppd[ds(64 * hi, 64), off:512],
                            vA[:, sc, HPC, :],
                            psb[:, hi, off:512],
                            start=first,
                            stop=last,
                            tile_position=(0, 64 * hi),
                            skip_group_check=True,
                        )

                # alternate bank roles between sweeps: the next sweep's
                # first y-MMs then wait on the bank freed by the recip
                # (early) rather than by the norm multiply (late)
                if (2 * tj + hp) % 2 == 0:
                    ppy = ps_y.tile([128, 512], F32, tag="yd")
                    ppd = ps_d.tile([128, 512], F32, tag="yd")
                else:
                    ppy = ps_d.tile([128, 512], F32, tag="yd")
                    ppd = ps_y.tile([128, 512], F32, tag="yd")
                # double-step batching: two QK steps, one fill unit, then
                # the two lagged PV steps — fewer psum-bank transitions
                LAG = 3 if n_sc > 4 else 2
                for i in range(0, n_sc, 2):
                    emit_qk(i)
                    if i + 1 < n_sc:
                        emit_qk(i + 1)
                    # extra fill right after a sweep boundary: the first
                    # PV waits on the previous sweep's norm (ps_y reuse)
                    pop_fill(2 if i == 0 else 1)
                    for j in (i - LAG, i + 1 - LAG):
                        if 0 <= j <= i + 1 - LAG:
                            emit_pv(j, ppy, ppd)
                for i in range(max(n_sc - LAG, 0), n_sc):
                    pop_fill(1)
                    emit_pv(i, ppy, ppd)
                # normalize: yT = y * (1/den), lane-aligned
                rc = prc.tile([128, 512], F32, tag="rc")
                nc.vector.reciprocal_approx_fast(rc[:], ppd[:])
                nc.vector.tensor_tensor(
                    yT[:, hp, ts(tj, 512)], ppy[:], rc[:], ALU.mult
                )

            # ---- schedule -------------------------------------------
            # prologue: projections for tj=0 (dense, PE warm by now)
            for hp in range(2):
                qk_proj_unit(wq_sb, bqs, qT, 0, hp, SCALE)()
                qk_proj_unit(wk_sb, bks, kT, 0, hp, None)()
            for sc in range(4):
                v_proj_unit(sc)()

            for tj in range(TJ):
                # load fill queue for this tj's attention sweeps
                if tj + 1 < TJ:
                    for hp in range(2):
                        fill.append(
                            qk_proj_unit(wq_sb, bqs, qT, tj + 1, hp, SCALE)
                        )
                        fill.append(
                            qk_proj_unit(wk_sb, bks, kT, tj + 1, hp, None)
                        )
                    for sc in range(4 * (tj + 1), 4 * (tj + 2)):
                        fill.append(v_proj_unit(sc))
                # o-proj fills split across the two hp sweeps so the
                # second sweep's PV-flush still has fill to hide stalls
                if tj >= 1:
                    for tt in range(2):
                        fill.append(o_proj_unit(tj - 1, tt))
                attn_sweep(tj, 0)
                if tj >= 1:
                    for tt in range(2, 4):
                        fill.append(o_proj_unit(tj - 1, tt))
                attn_sweep(tj, 1)

            # epilogue: drain remaining fill + last o-projection
            pop_fill(len(fill))
            for tt in range(4):
                o_proj_unit(TJ - 1, tt)()

    nc.compile()
    return nc


_CACHE = {}


def _get_program():
    if "nc" not in _CACHE:
        _CACHE["nc"] = build_program()
    return _CACHE["nc"]


def make_in_maps(x, wq, bq, wk, bk, wv, bv, wo):
    bf = mybir.dt.np(BF16)
    xb_ = np.asarray(x, np.float32).astype(bf)
    wqb = np.asarray(wq, np.float32).astype(bf)
    wkb = np.asarray(wk, np.float32).astype(bf)
    wvb = np.asarray(wv, np.float32).astype(bf)
    wob = np.asarray(wo, np.float32).astype(bf)
    in_maps = []
    for core in range(N_CORES):
        b, g = core // 4, core % 4
        sl = slice(g * DQC, (g + 1) * DQC)
        wqs = wqb[:, sl]
        wks = wkb[:, sl]
        in_maps.append(
            {
                "xbT": np.ascontiguousarray(xb_[b].T),
                "wq": np.ascontiguousarray(
                    np.stack([wqs[:, 0:128], wqs[:, 128:256]])
                ),
                "wk": np.ascontiguousarray(
                    np.stack([wks[:, 0:128], wks[:, 128:256]])
                ),
                "wv": np.ascontiguousarray(wvb[:, sl]),
                "wo": np.ascontiguousarray(wob[sl, :]),
                "bq": np.ascontiguousarray(np.asarray(bq, np.float32)[sl]),
                "bk": np.ascontiguousarray(np.asarray(bk, np.float32)[sl]),
                "bv": np.ascontiguousarray(np.asarray(bv, np.float32)[sl]),
            }
        )
    return in_maps


def kernel(x, wq, bq, wk, bk, wv, bv, wo, bo):
    from concourse import bass_utils

    bo = np.asarray(bo, dtype=np.float32)

    nc = _get_program()
    in_maps = make_in_maps(x, wq, bq, wk, bk, wv, bv, wo)
    res = bass_utils.run_bass_kernel_spmd(
        nc, in_maps, core_ids=list(range(N_CORES))
    )
    y = np.zeros((B, T, C), dtype=np.float32)
    for core in range(N_CORES):
        y[core // 4] += res.results[core]["out"]
    y += bo
    return y


# revision 41
# speedup vs baseline: 1.0894x; 1.0894x over previous
"""Multi-head masked attention on 8 Trainium2 NeuronCores.

Sharding: data-parallel over batch (B=2 -> 2 groups of 4 cores),
tensor-parallel over heads within a group (16 heads -> 4 heads/core).
Each core computes q/k/v projections for its 4 heads (column-sharded),
causal attention in the transposed (S^T) domain, and a row-sharded
partial o-projection. The host sums the 4 partials per batch element
and adds the output bias.

Pipeline structure: projection / o-projection matmuls are emitted as
fill units interleaved into the attention QK->exp->PV stream so the PE
never idles long enough for the HAM clock gate to re-throttle. PV uses
a merged [V | ones] stationary per head: one N-wide stream yields both
y^T (rows 0-63) and the softmax denominator replicated across rows
64-127, so the denominator costs no extra PE stream and normalization
is a partition-offset DVE multiply. The prologue streams x^T per
c-chunk so the first projection starts as soon as its slice lands.

Self-contained: hardcodes shapes B=2, T=2048, C=1024, H=16, Dh=64.
"""

import sys

sys.path.insert(0, "/opt/trn_rl_repo")

import numpy as np

import concourse.bass as bass
import concourse.tile as tile
import concourse.mybir as mybir
from concourse import bacc
from concourse.bass import ts, ds
from concourse.masks import make_identity, make_lower_triangular

F32 = mybir.dt.float32
BF16 = mybir.dt.bfloat16
AF = mybir.ActivationFunctionType
ALU = mybir.AluOpType

B, T, C = 2, 2048, 1024
H, DH = 16, 64
HPC = 4            # heads per core
DQC = HPC * DH     # 256 projected dims per core
N_CORES = 8
NEG = -1.0e30

TC = T // 128      # 16 s-chunks of 128
CC = C // 128      # 8 c-chunks
TJ = T // 512      # 4 t-chunks of 512
SCALE = 1.0 / np.sqrt(DH)


def build_program():
    nc = bacc.Bacc("TRN2", target_bir_lowering=False, debug=False)

    xbT = nc.dram_tensor("xbT", [C, T], BF16, kind="ExternalInput")
    wq = nc.dram_tensor("wq", [2, C, 128], BF16, kind="ExternalInput")
    wk = nc.dram_tensor("wk", [2, C, 128], BF16, kind="ExternalInput")
    wv = nc.dram_tensor("wv", [C, DQC], BF16, kind="ExternalInput")
    wo = nc.dram_tensor("wo", [DQC, C], BF16, kind="ExternalInput")
    bq = nc.dram_tensor("bq", [DQC], F32, kind="ExternalInput")
    bk = nc.dram_tensor("bk", [DQC], F32, kind="ExternalInput")
    bv = nc.dram_tensor("bv", [DQC], F32, kind="ExternalInput")
    out = nc.dram_tensor("out", [T, C], BF16, kind="ExternalOutput")

    with tile.TileContext(nc) as tc:
        with (
            tc.tile_pool(name="persist", bufs=1) as pp,
            tc.tile_pool(name="psb", bufs=5) as pexp,
            tc.tile_pool(name="otp", bufs=3) as pot,
            tc.tile_pool(name="rcp", bufs=2) as prc,
            tc.tile_pool(name="ps_s", bufs=2, space="PSUM") as ps_s,
            tc.tile_pool(name="ps_y", bufs=1, space="PSUM") as ps_y,
            tc.tile_pool(name="ps_d", bufs=1, space="PSUM") as ps_d,
            tc.tile_pool(name="ps_po", bufs=2, space="PSUM") as ps_po,
        ):
            # ---- persistent sbuf tensors -------------------------------
            xT = pp.tile([128, CC, T], BF16, tag="xT")
            qT = pp.tile([128, 2, T], BF16, tag="qT")   # [d-pair, hp, t]
            kT = pp.tile([128, 2, T], BF16, tag="kT")
            # [s, sc, h, d]; slot HPC holds all-ones denominator columns
            vA = pp.tile([128, TC, HPC + 1, DH], BF16, tag="vA")
            yT = pp.tile([128, 2, T], BF16, tag="yT")
            wq_sb = pp.tile([128, CC, DQC], BF16, tag="wq")
            wk_sb = pp.tile([128, CC, DQC], BF16, tag="wk")
            wv_sb = pp.tile([128, CC, DQC], BF16, tag="wv")
            wo_sb = pp.tile([128, 2, C], BF16, tag="wo")
            identb = pp.tile([128, 128], BF16, tag="identb")
            atrif = pp.tile([128, 128], BF16, tag="atrif")
            atrif2 = pp.tile([128, 2, 128], F32, tag="atrif2")
            warm = pp.tile([128, 128], BF16, tag="warm")
            bqs = pp.tile([128, 2], F32, tag="bqs")
            bks = pp.tile([128, 2], F32, tag="bks")
            bvs = pp.tile([128, DQC], F32, tag="bvs")

            # ---- PE warmup: keep the array busy while DMA lands -------
            # (memset on DVE: the Pool engine's first instruction pays a
            # multi-us ucode bring-up that would delay the warm matmuls)
            nc.vector.memset(warm[:], 0.25)
            pw0 = ps_po.tile([128, 512], F32, tag="po")
            for i in range(36):
                nc.tensor.matmul(
                    pw0[:, :128], warm[:], warm[:],
                    start=True, stop=True, skip_group_check=True,
                )

            # ---- DMA: first-needed pieces first -----------------------
            xTr = xbT.ap().rearrange("(c p) t -> p c t", p=128)
            nc.sync.dma_start(
                wq_sb[:, :, 0:128],
                wq.ap()[0].rearrange("(c p) d -> p c d", p=128),
            )
            nc.sync.dma_start(
                wk_sb[:, :, 0:128],
                wk.ap()[0].rearrange("(c p) d -> p c d", p=128),
            )
            # biases early: the first projection drains need them
            nc.sync.dma_start(bqs[:], bq.ap().rearrange("(k p) -> p k", p=128))
            nc.vector.tensor_scalar_mul(bqs[:], bqs[:], SCALE)
            nc.sync.dma_start(bks[:], bk.ap().rearrange("(k p) -> p k", p=128))
            nc.sync.dma_start(
                bvs[0:1, :], bv.ap().rearrange("(o n) -> o n", o=1)
            )
            nc.gpsimd.partition_broadcast(bvs[:], bvs[0:1, :])
            # Input DMAs are spread across three rings (sync / gpsimd /
            # scalar HWDGE) — a single ring sustains only ~115 GB/s, so
            # serializing the ~6 MB of inputs on one ring starves the
            # prologue. Each ring gets its pieces in first-needed order.
            # sync ring: x^T chunk 0 low c-slices (Q(0,h0) starts at
            # slice-0 landing)
            for cc in range(CC // 2):
                nc.sync.dma_start(
                    xT[:, cc, ts(0, 512)], xTr[:, cc, ts(0, 512)]
                )
            # gpsimd ring: second-half weights, x0 high c-slices, wv, x3
            nc.gpsimd.dma_start(
                wq_sb[:, :, 128:256],
                wq.ap()[1].rearrange("(c p) d -> p c d", p=128),
            )
            nc.gpsimd.dma_start(
                wk_sb[:, :, 128:256],
                wk.ap()[1].rearrange("(c p) d -> p c d", p=128),
            )
            for cc in range(CC // 2, CC):
                nc.gpsimd.dma_start(
                    xT[:, cc, ts(0, 512)], xTr[:, cc, ts(0, 512)]
                )
            nc.gpsimd.dma_start(
                wv_sb[:], wv.ap().rearrange("(c p) d -> p c d", p=128)
            )
            nc.gpsimd.dma_start(
                xT[:, :, ts(3, 512)], xTr[:, :, ts(3, 512)]
            )
            # scalar ring: x1, x2, wo
            for tj in range(1, 3):
                nc.scalar.dma_start(
                    xT[:, :, ts(tj, 512)], xTr[:, :, ts(tj, 512)]
                )
            nc.scalar.dma_start(
                wo_sb[:], wo.ap().rearrange("(k p) n -> p k n", p=128)
            )

            # constants
            make_identity(nc, identb[:])
            # atrif[p, f] = NEG where f < p (kills s>t in a diagonal block)
            make_lower_triangular(nc, atrif[:], val=NEG, diag=False)
            # f32 copy replicated for both head-halves (DVE mask add)
            nc.vector.tensor_copy(atrif2[:, 0, :], atrif[:])
            nc.vector.tensor_copy(atrif2[:, 1, :], atrif[:])
            # denominator ones-columns (slot HPC of vA)
            nc.vector.memset(vA[:, :, HPC, :], 1.0)

            # ---- work units (each emits a short burst of PE work) -----
            def qk_proj_unit(w_sb, b_sb, dstT, tj, hp, pre_scale):
                def emit():
                    pq = ps_po.tile([128, 512], F32, tag="po")
                    for cc in range(CC):
                        nc.tensor.matmul(
                            pq[:],
                            w_sb[:, cc, ts(hp, 128)],
                            xT[:, cc, ts(tj, 512)],
                            start=(cc == 0),
                            stop=(cc == CC - 1),
                        )
                    if pre_scale is not None:
                        nc.vector.tensor_scalar(
                            dstT[:, hp, ts(tj, 512)],
                            pq[:], pre_scale, b_sb[:, hp : hp + 1],
                            ALU.mult, ALU.add,
                        )
                    else:
                        nc.vector.tensor_scalar(
                            dstT[:, hp, ts(tj, 512)],
                            pq[:], b_sb[:, hp : hp + 1], None, ALU.add,
                        )
                return emit

            def v_proj_unit(sc):
                def emit():
                    pv = ps_po.tile([128, 512], F32, tag="po")
                    for cc in range(CC):
                        nc.tensor.matmul(
                            pv[:, :DQC],
                            xT[:, cc, ts(sc, 128)],
                            wv_sb[:, cc, :],
                            start=(cc == 0),
                            stop=(cc == CC - 1),
                        )
                    nc.vector.tensor_tensor(
                        vA[:, sc, 0:HPC, :],
                        pv[:, :DQC].rearrange("p (h d) -> p h d", d=DH),
                        bvs[:].rearrange("p (h d) -> p h d", d=DH),
                        ALU.add,
                    )
                return emit

            def o_proj_unit(tj, tt):
                # one t-block of the o-projection: out[t0:t0+128, :];
                # output DMAs alternate rings so the final stores overlap
                def emit():
                    t0 = 512 * tj + 128 * tt
                    ot = pot.tile([128, C], BF16, tag="ot")
                    for nb in range(2):
                        po = ps_po.tile([128, 512], F32, tag="po")
                        for kk in range(2):
                            nc.tensor.matmul(
                                po[:],
                                yT[:, kk, ds(t0, 128)],
                                wo_sb[:, kk, ts(nb, 512)],
                                start=(kk == 0),
                                stop=(kk == 1),
                            )
                        nc.vector.tensor_copy(ot[:, ts(nb, 512)], po[:])
                    eng = nc.sync if tt % 2 == 0 else nc.scalar
                    eng.dma_start(out.ap()[ds(t0, 128), :], ot[:])
                return emit

            fill = []

            def pop_fill(n=1):
                for _ in range(n):
                    if fill:
                        fill.pop(0)()

            # ---- attention sweep for one (tj, hp) head pair ------------
            def attn_sweep(tj, hp):
                n_sc = 4 * (tj + 1)
                psbs = {}

                def emit_qk(sc):
                    k = sc - 4 * tj
                    off = 128 * k if k > 0 else 0
                    pss = ps_s.tile([128, 2, 512], F32, tag="s")
                    for hi in range(2):
                        prow = slice(64 * hi, 64 * hi + 64)
                        nc.tensor.matmul(
                            pss[:, hi, off:512],
                            kT[prow, hp, ts(sc, 128)],
                            qT[prow, hp, ds(512 * tj + off, 512 - off)],
                            start=True,
                            stop=True,
                            tile_position=(64 * hi, 0),
                            skip_group_check=(k >= 0),
                        )
                    if k >= 0:
                        # causal mask on the diagonal block: DVE add keeps
                        # the PE free of identb weight-switch flushes
                        nc.vector.tensor_tensor(
                            pss[:, :, ds(off, 128)],
                            pss[:, :, ds(off, 128)],
                            atrif2[:],
                            ALU.add,
                        )
                    psb = pexp.tile([128, 2, 512], BF16, tag="p")
                    if off:
                        nc.scalar.activation(
                            psb[:, :, off:], pss[:, :, off:], AF.Exp
                        )
                    else:
                        nc.scalar.activation(psb[:], pss[:], AF.Exp)
                    psbs[sc] = psb

                def emit_pv(sc, ppy, ppd):
                    k = sc - 4 * tj
                    off = 128 * k if k > 0 else 0
                    psb = psbs.pop(sc)
                    first = sc == 0
                    last = sc == n_sc - 1
                    # y pair first (col groups 0-1 / 2-3 run concurrent),
                    # then the denominator pair
                    for hi in range(2):
                        nc.tensor.matmul(
                            ppy[ds(64 * hi, 64), off:512],
                            vA[:, sc, 2 * hp + hi, :],
                            psb[:, hi, off:512],
                            start=first,
                            stop=last,
                            tile_position=(0, 64 * hi),
                            skip_group_check=True,
                        )
                    for hi in range(2):
                        nc.tensor.matmul(
                            ppd[ds(64 * hi, 64), off:512],
                            vA[:, sc, HPC, :],
                            psb[:, hi, off:512],
                            start=first,
                            stop=last,
                            tile_position=(0, 64 * hi),
                            skip_group_check=True,
                        )

                # alternate bank roles between sweeps: the next sweep's
                # first y-MMs then wait on the bank freed by the recip
                # (early) rather than by the norm multiply (late)
                if (2 * tj + hp) % 2 == 0:
                    ppy = ps_y.tile([128, 512], F32, tag="yd")
                    ppd = ps_d.tile([128, 512], F32, tag="yd")
                else:
                    ppy = ps_d.tile([128, 512], F32, tag="yd")
                    ppd = ps_y.tile([128, 512], F32, tag="yd")
                # double-step batching: two QK steps, one fill unit, then
                # the two lagged PV steps — fewer psum-bank transitions
                LAG = 3 if n_sc > 4 else 2
                for i in range(0, n_sc, 2):
                    emit_qk(i)
                    if i + 1 < n_sc:
                        emit_qk(i + 1)
                    # extra fill right after a sweep boundary: the first
                    # PV waits on the previous sweep's norm (ps_y reuse)
                    pop_fill(2 if i == 0 else 1)
                    for j in (i - LAG, i + 1 - LAG):
                        if 0 <= j <= i + 1 - LAG:
                            emit_pv(j, ppy, ppd)
                for i in range(max(n_sc - LAG, 0), n_sc):
                    pop_fill(1)
                    emit_pv(i, ppy, ppd)
                # normalize: yT = y * (1/den), lane-aligned
                rc = prc.tile([128, 512], F32, tag="rc")
                nc.vector.reciprocal_approx_fast(rc[:], ppd[:])
                nc.vector.tensor_tensor(
                    yT[:, hp, ts(tj, 512)], ppy[:], rc[:], ALU.mult
                )

            # ---- schedule -------------------------------------------
            # prologue: projections for tj=0 (dense, PE warm by now)
            for hp in range(2):
                qk_proj_unit(wq_sb, bqs, qT, 0, hp, SCALE)()
                qk_proj_unit(wk_sb, bks, kT, 0, hp, None)()
            for sc in range(4):
                v_proj_unit(sc)()

            for tj in range(TJ):
                # load fill queue for this tj's attention sweeps
                if tj + 1 < TJ:
                    for hp in range(2):
                        fill.append(
                            qk_proj_unit(wq_sb, bqs, qT, tj + 1, hp, SCALE)
                        )
                        fill.append(
                            qk_proj_unit(wk_sb, bks, kT, tj + 1, hp, None)
                        )
                    for sc in range(4 * (tj + 1), 4 * (tj + 2)):
                        fill.append(v_proj_unit(sc))
                # o-proj fills split across the two hp sweeps so the
                # second sweep's PV-flush still has fill to hide stalls
                if tj >= 1:
                    for tt in range(2):
                        fill.append(o_proj_unit(tj - 1, tt))
                attn_sweep(tj, 0)
                if tj >= 1:
                    for tt in range(2, 4):
                        fill.append(o_proj_unit(tj - 1, tt))
                attn_sweep(tj, 1)

            # epilogue: drain remaining fill + last o-projection
            pop_fill(len(fill))
            for tt in range(4):
                o_proj_unit(TJ - 1, tt)()

    nc.compile()
    return nc


_CACHE = {}


def _get_program():
    if "nc" not in _CACHE:
        _CACHE["nc"] = build_program()
    return _CACHE["nc"]


def make_in_maps(x, wq, bq, wk, bk, wv, bv, wo):
    bf = mybir.dt.np(BF16)
    xb_ = np.asarray(x, np.float32).astype(bf)
    wqb = np.asarray(wq, np.float32).astype(bf)
    wkb = np.asarray(wk, np.float32).astype(bf)
    wvb = np.asarray(wv, np.float32).astype(bf)
    wob = np.asarray(wo, np.float32).astype(bf)
    in_maps = []
    for core in range(N_CORES):
        b, g = core // 4, core % 4
        sl = slice(g * DQC, (g + 1) * DQC)
        wqs = wqb[:, sl]
        wks = wkb[:, sl]
        in_maps.append(
            {
                "xbT": np.ascontiguousarray(xb_[b].T),
                "wq": np.ascontiguousarray(
                    np.stack([wqs[:, 0:128], wqs[:, 128:256]])
                ),
                "wk": np.ascontiguousarray(
                    np.stack([wks[:, 0:128], wks[:, 128:256]])
                ),
                "wv": np.ascontiguousarray(wvb[:, sl]),
                "wo": np.ascontiguousarray(wob[sl, :]),
                "bq": np.ascontiguousarray(np.asarray(bq, np.float32)[sl]),
                "bk": np.ascontiguousarray(np.asarray(bk, np.float32)[sl]),
                "bv": np.ascontiguousarray(np.asarray(bv, np.float32)[sl]),
            }
        )
    return in_maps


def kernel(x, wq, bq, wk, bk, wv, bv, wo, bo):
    from concourse import bass_utils

    bo = np.asarray(bo, dtype=np.float32)

    nc = _get_program()
    in_maps = make_in_maps(x, wq, bq, wk, bk, wv, bv, wo)
    res = bass_utils.run_bass_kernel_spmd(
        nc, in_maps, core_ids=list(range(N_CORES))
    )
    y = np.zeros((B, T, C), dtype=np.float32)
    for core in range(N_CORES):
        y[core // 4] += res.results[core]["out"]
    y += bo
    return y
